# revision 25
# baseline (speedup 1.0000x reference)
"""Trainium2 Bass kernel for nn_LocalFeatureExtractor (gnn_message_passing).

Math: with per-node features x[b,n,:] (C=128) and K=10 gathered neighbors,
    out = x @ W1^T + W1_b + (conv(feats) + Wc_b) @ W2^T + W2_b
collapses algebraically (fold the two dense layers around the conv) to
    out[b,n] = x[b,n] @ A + sum_k x[b, adj[b,n,k]] @ M_k + bias
with A = W1^T + (W2 Wc_0)^T, M_k = (W2 Wc_k)^T, bias = W1_b + W2_b + W2 Wc_b.

Sharding: data-parallel over batch B=8 -> one graph per NeuronCore.

The 200K per-core column gathers run on the 16 DMA engines via SWDGE
dma_gather (transpose mode): each int16 index pulls one 256B bf16 row of x
straight out of DRAM and transposes it into matmul-rhs orientation
[C partitions, nodes] in SBUF. GPSIMD only generates descriptors
(~0.34ns/idx), so gather throughput is DMA-bus-bound (~1.42ns/column)
instead of GPSIMD-bound (~28us per gather instruction in the ap_gather
formulation). Matmuls are bf16 (tolerance 2e-2), accumulate in fp32 PSUM.
"""

import numpy as np

import concourse.bass as bass
import concourse.mybir as mybir
from concourse import bacc
from concourse.tile import TileContext
from concourse.bass_utils import run_bass_kernel_spmd

B, N, C, K = 8, 20000, 128, 10
N_CORES = 8

TGD = 250                    # nodes per gather group (= psum chunk)
GD = N // TGD                # 80 groups
KSPLIT = 1                   # gather instructions per group (k-slots split)
KPG = K // KSPLIT            # 5 neighbor slots per gather instruction
NDV = KPG * TGD              # 2500 valid idxs per dma_gather
NID = -(-NDV // 128) * 128   # padded to 2560 (transpose needs %128==0)
CHUNK = 500                  # psum tile columns (<=512 fp32)

_dt = mybir.dt


def build(n_cores=N_CORES, reps=1, n=N, tgd=TGD, bbufs=3, psbufs=4, obufs=4,
          ksplit=KSPLIT, ob=4):
    """Build + compile the per-core Bass program (SPMD: same program, 8 cores)."""
    bf16 = _dt.bfloat16
    gd = n // tgd
    kpg = K // ksplit        # neighbor slots per gather instruction
    ndv = kpg * tgd
    nid = -(-ndv // 128) * 128
    chunk = min(CHUNK, tgd)
    cpg = tgd // chunk       # psum chunks per gather group

    nc = bacc.Bacc("TRN2", target_bir_lowering=False, debug=False,
                   num_devices=n_cores)
    xT = nc.dram_tensor("xT", [C, n], bf16, kind="ExternalInput").ap()
    xr = nc.dram_tensor("xr", [n, C], bf16, kind="ExternalInput").ap()
    idxd = nc.dram_tensor("idxd", [C, gd * ksplit * (nid // 16)], _dt.int16,
                          kind="ExternalInput").ap()
    wts = nc.dram_tensor("wts", [C, (K + 1) * C], bf16, kind="ExternalInput").ap()
    bias = nc.dram_tensor("bias", [C, 1], _dt.float32, kind="ExternalInput").ap()
    outT = nc.dram_tensor("outT", [C, n], bf16, kind="ExternalOutput").ap()

    with TileContext(nc) as tc:
        with tc.tile_pool(name="const", bufs=1) as cpool, \
             tc.tile_pool(name="gd_pool", bufs=bbufs) as bpool, \
             tc.tile_pool(name="psum", bufs=psbufs, space="PSUM") as ppool, \
             tc.tile_pool(name="outp", bufs=obufs) as opool:
            xT_t = cpool.tile([C, n], bf16)
            idxd_t = cpool.tile([C, gd * ksplit * (nid // 16)], _dt.int16)
            wts_t = cpool.tile([C, (K + 1) * C], bf16)
            bias_t = cpool.tile([C, 1], _dt.float32)
            nc.sync.dma_start(out=idxd_t[:], in_=idxd[:])
            nc.sync.dma_start(out=xT_t[:], in_=xT[:])
            nc.sync.dma_start(out=wts_t[:], in_=wts[:])
            nc.sync.dma_start(out=bias_t[:], in_=bias[:])

            for _rep in range(reps):
                for g in range(gd):
                    bts = []
                    for h in range(ksplit):
                        bt = bpool.tile([C, 1, nid], bf16, tag="b%d" % h,
                                        name="b%d_%d_%d" % (_rep, g, h))
                        s = (g * ksplit + h) * (nid // 16)
                        nc.gpsimd.dma_gather(
                            out_ap=bt[:],
                            in_ap=xr[:],
                            idxs_ap=idxd_t[:, s:s + nid // 16],
                            num_idxs=nid,
                            num_idxs_reg=ndv,
                            elem_size=C,
                            transpose=True,
                            single_packet=False,
                        )
                        bts.append(bt)
                    for cc in range(cpg):
                        c = g * cpg + cc
                        ps = ppool.tile([C, chunk], _dt.float32, tag="ps",
                                        name="ps%d_%d" % (_rep, c))
                        nc.tensor.matmul(
                            out=ps[:],
                            lhsT=wts_t[:, 0:C],
                            rhs=xT_t[:, c * chunk:(c + 1) * chunk],
                            start=True, stop=False,
                        )
                        for k in range(1, K + 1):
                            h, kl = (k - 1) // kpg, (k - 1) % kpg
                            off = kl * tgd + cc * chunk
                            nc.tensor.matmul(
                                out=ps[:],
                                lhsT=wts_t[:, k * C:(k + 1) * C],
                                rhs=bts[h][:, 0, off:off + chunk],
                                start=False, stop=(k == K),
                            )
                        nch = n // chunk
                        if c % ob == 0:
                            o_cur = opool.tile([C, min(ob, nch - c) * chunk],
                                               bf16, tag="o",
                                               name="o%d_%d" % (_rep, c))
                        oc = c % ob
                        nc.scalar.activation(
                            o_cur[:, oc * chunk:(oc + 1) * chunk], ps[:],
                            mybir.ActivationFunctionType.Identity,
                            bias=bias_t[:], scale=1.0,
                        )
                        if oc == ob - 1 or c == nch - 1:
                            c0 = c - oc
                            nc.sync.dma_start(
                                out=outT[:, c0 * chunk:(c + 1) * chunk],
                                in_=o_cur[:, 0:(oc + 1) * chunk])
    nc.compile()
    return nc


def fold_weights(W1_w, W1_b, Wc_w, Wc_b, W2_w, W2_b):
    """Collapse Linear->Conv1d->Linear into 11 [C,C] mats + one bias."""
    W2 = W2_w.astype(np.float64)
    M = np.einsum('de,eck->cdk', W2, Wc_w.astype(np.float64))
    M[:, :, 0] += W1_w.T.astype(np.float64)
    wts = np.concatenate([M[:, :, k] for k in range(K + 1)], axis=1)
    bias = W1_b.astype(np.float64) + W2_b.astype(np.float64) + W2 @ Wc_b.astype(np.float64)
    return wts.astype(np.float32), bias.astype(np.float32).reshape(C, 1)


def make_idx_dma(adj_b):
    """dma_gather idx tensor [128, GD*KSPLIT*(NID//16)] int16: per 500-node
    group and k-half, k-major (j = k_local*TGD + i), padded to NID with -1,
    wrapped so idx j sits at (partition j%16, slot j//16), 16-row block
    replicated to 128 rows."""
    a = np.asarray(adj_b).reshape(GD, TGD, K)
    blocks = []
    for g in range(GD):
        for h in range(KSPLIT):
            arr = a[g, :, h * KPG:(h + 1) * KPG]          # [TGD, KPG]
            jf = np.full(NID, -1, dtype=np.int16)
            jf[:NDV] = arr.T.reshape(-1)                  # k-major
            blocks.append(jf.reshape(NID // 16, 16).T)    # [16, slots]
    blk = np.concatenate(blocks, axis=1)
    return np.tile(blk, (8, 1)).copy()


def prep_core_inputs(x, adj_mat, wts, bias):
    """Per-core (per-graph) input maps for the SPMD launch."""
    bf16 = _dt.np(_dt.bfloat16)
    wts_bf = np.ascontiguousarray(wts.astype(bf16))
    maps = []
    for b in range(B):
        xr = np.ascontiguousarray(np.asarray(x[b]).astype(bf16))
        maps.append({
            "xT": np.ascontiguousarray(xr.T),
            "xr": xr,
            "idxd": make_idx_dma(adj_mat[b]),
            "wts": wts_bf,
            "bias": bias.astype(np.float32),
        })
    return maps


_NC_CACHE = {}


def kernel(x, adj_mat, W1_w, W1_b, Wc_w, Wc_b, W2_w, W2_b):
    x = np.asarray(x)
    adj_mat = np.asarray(adj_mat)
    wts, bias = fold_weights(np.asarray(W1_w), np.asarray(W1_b), np.asarray(Wc_w),
                             np.asarray(Wc_b), np.asarray(W2_w), np.asarray(W2_b))
    if "nc" not in _NC_CACHE:
        _NC_CACHE["nc"] = build()
    nc = _NC_CACHE["nc"]
    in_maps = prep_core_inputs(x, adj_mat, wts, bias)
    res = run_bass_kernel_spmd(nc, in_maps, list(range(N_CORES)))
    out = np.empty((B, N, C), dtype=np.float32)
    for b in range(B):
        out[b] = res.results[b]["outT"].T.astype(np.float32)
    return out
